# revision 1
# baseline (speedup 1.0000x reference)
"""Trainium2 Bass kernel for GroupedQueryAttention (sparse sliding-window + global).

Sharding: 8 cores = 2 (batch) x 4 (GQA groups). Core c handles batch c//4 and
kv-head g=c%4 together with its 4 query heads (heads 4g..4g+3). Wq/Wk/Wv are
column-sharded, Wo row-sharded; each core emits a transposed partial output
outT = (context_g @ Wo_g)^T which the host transposes and sums per batch.
"""

import sys

for _p in (
    "/opt/trn_rl_repo",
    "/root/.axon_site",
    "/root/.axon_site/_ro/pypackages",
    "/root/.axon_site/_ro/trn_rl_repo",
):
    if _p not in sys.path:
        sys.path.insert(0, _p)

from contextlib import ExitStack

import numpy as np

import concourse.bass as bass  # noqa: F401  (registers engine classes)
import concourse.tile as tile
from concourse import bacc, mybir
from concourse.bass_utils import run_bass_kernel_spmd
from concourse.masks import make_identity

B, S, DM = 2, 2048, 1024
NH, NKV, DH = 16, 4, 64
HPC = 4  # q heads per core (one full GQA group)
WINDOW, NGLOB = 256, 4
SCALE = 1.0 / np.sqrt(DH)
CAP = 15.0
EPS = 1e-8
P = 128
NT = S // P  # 16 sequence tiles
G = HPC + 1  # 4 q heads + 1 k head share L2norm/RoPE processing
F32 = mybir.dt.float32
F32R = mybir.dt.float32r
BF16 = mybir.dt.bfloat16
MULT = mybir.AluOpType.mult


def _build_kernel(ctx, tc, d):
    nc = tc.nc

    consts = ctx.enter_context(tc.tile_pool(name="consts", bufs=1))
    ident = consts.tile([P, P], F32)
    make_identity(nc, ident[:])
    ident_bf = consts.tile([P, P], BF16)
    nc.vector.tensor_copy(ident_bf[:], ident[:])

    wqkv_sb = consts.tile([P, 8, 384], BF16)
    nc.sync.dma_start(wqkv_sb[:], d["wqkv"].rearrange("(c p) n -> p c n", p=P))
    wo_sb = consts.tile([P, 2, DM], BF16)
    nc.sync.dma_start(wo_sb[:], d["wo"].rearrange("(c p) n -> p c n", p=P))
    cos_sb = consts.tile([P, NT, 32], F32)
    nc.sync.dma_start(cos_sb[:], d["cos"].rearrange("(t p) n -> p t n", p=P))
    sin_sb = consts.tile([P, NT, 32], F32)
    nc.sync.dma_start(sin_sb[:], d["sin"].rearrange("(t p) n -> p t n", p=P))
    ones1 = consts.tile([P, 1], F32)
    nc.vector.memset(ones1[:], 1.0)

    # persistent per-s-chunk tensors
    qt_pool = ctx.enter_context(tc.tile_pool(name="qt", bufs=NT))
    kt_pool = ctx.enter_context(tc.tile_pool(name="kt", bufs=NT))
    v_pool = ctx.enter_context(tc.tile_pool(name="v", bufs=NT))
    ctx_pool = ctx.enter_context(tc.tile_pool(name="ctx", bufs=8))

    xp = ctx.enter_context(tc.tile_pool(name="xp", bufs=3))
    xtp = ctx.enter_context(tc.tile_pool(name="xtp", bufs=10))
    work = ctx.enter_context(tc.tile_pool(name="work", bufs=3))
    attn = ctx.enter_context(tc.tile_pool(name="attn", bufs=3))

    ps_t = ctx.enter_context(tc.tile_pool(name="ps_t", bufs=2, space="PSUM"))
    ps_mm = ctx.enter_context(tc.tile_pool(name="ps_mm", bufs=2, space="PSUM"))
    ps_sc = ctx.enter_context(tc.tile_pool(name="ps_sc", bufs=2, space="PSUM"))
    ps_cx = ctx.enter_context(tc.tile_pool(name="ps_cx", bufs=2, space="PSUM"))

    qtiles, ktiles, vtiles = [], [], []
    ctxt = [[None] * 4, [None] * 4]
    for c in range(2):
        for sc in range(4):
            ctile = ctx_pool.tile([P, 512], BF16, name=f"ctx_{c}_{sc}", tag="ctx")
            ctxt[c][sc] = ctile

    # ---------------- Phase A: QKV projection, L2 norm, RoPE, transposes ----
    for i in range(NT):
        x_sb = xp.tile([P, DM], F32, tag="x")
        nc.sync.dma_start(x_sb[:], d["xs"][P * i : P * (i + 1), :])
        xb = xp.tile([P, DM], BF16, tag="xb")
        nc.vector.tensor_copy(xb[:], x_sb[:])

        xts = []
        for mj in range(8):
            pt = ps_t.tile([P, P], BF16, name=f"ptx_{i}_{mj}", tag="t")
            nc.tensor.transpose(pt[:], xb[:, P * mj : P * (mj + 1)], ident_bf[:])
            xt = xtp.tile([P, P], BF16, name=f"xt_{i}_{mj}", tag="xt")
            if mj % 2 == 0:
                nc.scalar.copy(xt[:], pt[:])
            else:
                nc.vector.tensor_copy(xt[:], pt[:])
            xts.append(xt)

        pq = ps_mm.tile([P, 384], F32, name=f"pqkv_{i}", tag="mm")
        for mj in range(8):
            nc.tensor.matmul(
                pq[:],
                lhsT=xts[mj][:],
                rhs=wqkv_sb[:, mj, :],
                start=(mj == 0),
                stop=(mj == 7),
            )

        # L2 normalization over d for q heads and k head (first 320 cols)
        ssq = work.tile([P, G * DH], F32, tag="ssq")
        nc.scalar.square(ssq[:], pq[:, 0 : G * DH])
        red = work.tile([P, G], F32, tag="red")
        nc.vector.tensor_reduce(
            red[:],
            ssq[:].rearrange("p (g n) -> p g n", g=G),
            axis=mybir.AxisListType.X,
            op=mybir.AluOpType.add,
        )
        nrm = work.tile([P, G], F32, tag="nrm")
        nc.scalar.sqrt(nrm[:], red[:])
        nrm2 = work.tile([P, G], F32, tag="nrm2")
        nc.vector.tensor_scalar_add(nrm2[:], nrm[:], EPS)
        rcn = work.tile([P, G], F32, tag="rcn")
        nc.vector.reciprocal(rcn[:], nrm2[:])
        qkn = work.tile([P, G * DH], F32, tag="qkn")
        nc.vector.tensor_tensor(
            qkn[:].rearrange("p (g n) -> p g n", g=G),
            pq[:, 0 : G * DH].rearrange("p (g n) -> p g n", g=G),
            rcn[:].unsqueeze(-1).broadcast_to([P, G, DH]),
            op=MULT,
        )

        # v (+ ones column for softmax sums)
        vt_i = v_pool.tile([P, 65], BF16, name=f"v_{i}", tag="v")
        nc.scalar.copy(vt_i[:, 64:65], ones1[:])
        nc.scalar.copy(vt_i[:, 0:64], pq[:, 320:384])
        vtiles.append(vt_i)

        # RoPE: rotate halves (d, d+32) with cos/sin of this s-chunk
        qv = qkn[:].rearrange("p (g n) -> p g n", g=G)
        x1, x2 = qv[:, :, 0:32], qv[:, :, 32:64]
        cb = cos_sb[:, i, :].unsqueeze(1).broadcast_to([P, G, 32])
        sbr = sin_sb[:, i, :].unsqueeze(1).broadcast_to([P, G, 32])
        rp = work.tile([P, G * DH], BF16, tag="rp")
        rv = rp[:].rearrange("p (g n) -> p g n", g=G)
        ta = work.tile([P, G * 32], F32, tag="ta")
        tb = work.tile([P, G * 32], F32, tag="tb")
        tav = ta[:].rearrange("p (g n) -> p g n", g=G)
        tbv = tb[:].rearrange("p (g n) -> p g n", g=G)
        nc.vector.tensor_tensor(tav, x1, cb, op=MULT)
        nc.vector.tensor_tensor(tbv, x2, sbr, op=MULT)
        nc.vector.tensor_sub(rv[:, :, 0:32], tav, tbv)
        nc.vector.tensor_tensor(tav, x1, sbr, op=MULT)
        nc.vector.tensor_tensor(tbv, x2, cb, op=MULT)
        nc.vector.tensor_add(rv[:, :, 32:64], tav, tbv)

        # transpose q (2x 128-col blocks = 4 heads) and k (64 cols)
        qt_i = qt_pool.tile([64, HPC * P], BF16, name=f"qt_{i}", tag="qt")
        for hp in range(2):
            ptq = ps_t.tile([P, P], BF16, name=f"ptq_{i}_{hp}", tag="t")
            nc.tensor.transpose(ptq[:], rp[:, P * hp : P * (hp + 1)], ident_bf[:])
            nc.scalar.copy(qt_i[:, (2 * hp) * P : (2 * hp) * P + P], ptq[0:64, :])
            nc.vector.tensor_copy(
                qt_i[:, (2 * hp + 1) * P : (2 * hp + 1) * P + P], ptq[64:128, :]
            )
        ptk = ps_t.tile([P, P], BF16, name=f"ptk_{i}", tag="t")
        nc.tensor.transpose(ptk[0:64, :], rp[:, 256:320], ident_bf[:])
        kt_i = kt_pool.tile([64, P], BF16, name=f"kt_{i}", tag="kt")
        nc.scalar.copy(kt_i[:], ptk[0:64, :])
        qtiles.append(qt_i)
        ktiles.append(kt_i)

    # ---------------- Phase B: banded attention --------------------------
    for t in range(NT):
        kts = list(range(max(0, t - 2), t + 1))
        mb = attn.tile([P, 3, P], BF16, tag="mb")
        nc.sync.dma_start(mb[:], d["band"][t])
        qrhs = qtiles[t][:].rearrange("p (h q) -> p h q", h=HPC)
        pcx = ps_cx.tile([65, 512], F32, name=f"pcx_{t}", tag="cx")

        for j_, kt in enumerate(kts):
            j = kt - (t - 2)
            ps = ps_sc.tile([P, 512], F32, name=f"psc_{t}_{kt}", tag="sc")
            nc.tensor.matmul(
                ps[:], lhsT=ktiles[kt][:], rhs=qrhs, start=True, stop=True
            )
            ex = attn.tile([P, 512], BF16, tag="ex")
            nc.scalar.activation(
                ex[:], ps[:], mybir.ActivationFunctionType.Exp, scale=SCALE
            )
            em = attn.tile([P, 512], BF16, tag="em")
            nc.vector.tensor_tensor(
                em[:].rearrange("p (h q) -> p h q", h=HPC),
                ex[:].rearrange("p (h q) -> p h q", h=HPC),
                mb[:, j, :].unsqueeze(1).broadcast_to([P, HPC, P]),
                op=MULT,
            )
            nc.tensor.matmul(
                pcx[:],
                lhsT=vtiles[kt][:],
                rhs=em[:],
                start=(j_ == 0),
                stop=(j_ == len(kts) - 1 and t < 3),
            )

        if t >= 3:
            gm = attn.tile([4, P], BF16, tag="gm")
            nc.sync.dma_start(gm[:], d["glob"][t])
            psg = ps_sc.tile([4, 512], F32, name=f"psg_{t}", tag="sc")
            nc.tensor.matmul(
                psg[:], lhsT=ktiles[0][:, 0:4], rhs=qrhs, start=True, stop=True
            )
            exg = attn.tile([4, 512], BF16, tag="exg")
            nc.scalar.activation(
                exg[:], psg[:], mybir.ActivationFunctionType.Exp, scale=SCALE
            )
            emg = attn.tile([4, 512], BF16, tag="emg")
            nc.vector.tensor_tensor(
                emg[:].rearrange("p (h q) -> p h q", h=HPC),
                exg[:].rearrange("p (h q) -> p h q", h=HPC),
                gm[:].unsqueeze(1).broadcast_to([4, HPC, P]),
                op=MULT,
            )
            nc.tensor.matmul(
                pcx[:],
                lhsT=vtiles[0][0:4, :],
                rhs=emg[:],
                start=False,
                stop=True,
            )

        # softmax denominators (row 64 of pcx) -> reciprocal -> broadcast
        sm = attn.tile([1, 512], F32, tag="sm")
        nc.scalar.copy(sm[:], pcx[64:65, :])
        rb = attn.tile([64, 512], F32, tag="rb")
        nc.gpsimd.partition_broadcast(rb[:], sm[:])
        rc = attn.tile([64, 512], F32, tag="rc")
        nc.vector.reciprocal(rc[:], rb[:])

        sc_, qoff = t // 4, (t % 4) * P
        for h in range(HPC):
            c, p0 = h // 2, 64 * (h % 2)
            nc.vector.tensor_tensor(
                ctxt[c][sc_][p0 : p0 + 64, qoff : qoff + P],
                pcx[0:64, h * P : (h + 1) * P],
                rc[:, h * P : (h + 1) * P],
                op=MULT,
            )

    # ---------------- Phase C: output projection (transposed) ------------
    outp = ctx.enter_context(tc.tile_pool(name="outp", bufs=4))
    for sc in range(4):
        for mo in range(8):
            po = ps_mm.tile([P, 512], F32, name=f"po_{sc}_{mo}", tag="mm")
            for c in range(2):
                nc.tensor.matmul(
                    po[:],
                    lhsT=wo_sb[:, c, P * mo : P * (mo + 1)],
                    rhs=ctxt[c][sc][:],
                    start=(c == 0),
                    stop=(c == 1),
                )
            ob = outp.tile([P, 512], F32, tag="ob")
            if mo % 2 == 0:
                nc.scalar.copy(ob[:], po[:])
            else:
                nc.vector.tensor_copy(ob[:], po[:])
            nc.sync.dma_start(
                d["outT"][P * mo : P * (mo + 1), 512 * sc : 512 * (sc + 1)], ob[:]
            )


def build_program():
    nc = bacc.Bacc("TRN2", target_bir_lowering=False, debug=False, num_devices=8)
    d = {}
    d["xs"] = nc.dram_tensor("xs", [S, DM], F32, kind="ExternalInput").ap()
    d["wqkv"] = nc.dram_tensor("wqkv", [DM, 384], BF16, kind="ExternalInput").ap()
    d["wo"] = nc.dram_tensor("wo", [256, DM], BF16, kind="ExternalInput").ap()
    d["cos"] = nc.dram_tensor("cos", [S, 32], F32, kind="ExternalInput").ap()
    d["sin"] = nc.dram_tensor("sin", [S, 32], F32, kind="ExternalInput").ap()
    d["band"] = nc.dram_tensor("band", [NT, P, 3, P], BF16, kind="ExternalInput").ap()
    d["glob"] = nc.dram_tensor("glob", [NT, 4, P], BF16, kind="ExternalInput").ap()
    d["outT"] = nc.dram_tensor("outT", [DM, S], F32, kind="ExternalOutput").ap()
    with tile.TileContext(nc) as tc, ExitStack() as ctx:
        _build_kernel(ctx, tc, d)
    nc.compile()
    return nc


def make_masks(mask_np):
    """Pack the combined (caller mask & sliding-window|global) mask into the
    banded [k, q]-oriented tiles the kernel consumes."""
    mask_np = np.asarray(mask_np).astype(bool)
    q = np.arange(S)[:, None]
    k = np.arange(S)[None, :]
    wmask = ((k <= q) & (k > q - WINDOW)) | (k < NGLOB)
    combT = (mask_np[0, 0] & wmask).T.astype(np.float32)  # [k, q]
    band = np.zeros((NT, P, 3, P), np.float32)
    glob = np.zeros((NT, 4, P), np.float32)
    for t in range(NT):
        for kt in range(max(0, t - 2), t + 1):
            j = kt - (t - 2)
            band[t, :, j, :] = combT[P * kt : P * (kt + 1), P * t : P * (t + 1)]
        if t >= 3:
            glob[t] = combT[0:NGLOB, P * t : P * (t + 1)]
    return band, glob


def make_in_maps(x, cos, sin, mask, Wq, Wk, Wv, Wo):
    import ml_dtypes

    bf = ml_dtypes.bfloat16
    x, cos, sin = (np.asarray(a, np.float32) for a in (x, cos, sin))
    Wq, Wk, Wv, Wo = (np.asarray(a, np.float32).astype(bf) for a in (Wq, Wk, Wv, Wo))
    band, glob = make_masks(mask)
    band, glob = band.astype(bf), glob.astype(bf)
    in_maps = []
    for c in range(8):
        b, g = divmod(c, 4)
        wqkv = np.concatenate(
            [
                Wq[:, 256 * g : 256 * (g + 1)],
                Wk[:, 64 * g : 64 * (g + 1)],
                Wv[:, 64 * g : 64 * (g + 1)],
            ],
            axis=1,
        )
        in_maps.append(
            {
                "xs": np.ascontiguousarray(x[b]),
                "wqkv": np.ascontiguousarray(wqkv),
                "wo": np.ascontiguousarray(Wo[256 * g : 256 * (g + 1), :]),
                "cos": np.ascontiguousarray(cos),
                "sin": np.ascontiguousarray(sin),
                "band": band,
                "glob": glob,
            }
        )
    return in_maps


_PROGRAM = None


def _get_program():
    global _PROGRAM
    if _PROGRAM is None:
        _PROGRAM = build_program()
    return _PROGRAM


def kernel(x, cos, sin, mask, Wq, Wk, Wv, Wo, _trace=False, _trace_kwargs=None):
    nc = _get_program()
    in_maps = make_in_maps(x, cos, sin, mask, Wq, Wk, Wv, Wo)
    res = run_bass_kernel_spmd(
        nc, in_maps, list(range(8)), trace=_trace, **(_trace_kwargs or {})
    )
    out = np.zeros((B, S, DM), np.float32)
    for c in range(8):
        out[c // 4] += res.results[c]["outT"].T
    if _trace:
        kernel._last_results = res
    return out



# revision 4
# speedup vs baseline: 1.0853x; 1.0853x over previous
"""Trainium2 Bass kernel for GroupedQueryAttention (sparse sliding-window + global).

Sharding: 8 cores = 2 (batch) x 4 (GQA groups). Core c handles batch c//4 and
kv-head g=c%4 together with its 4 query heads (heads 4g..4g+3). Wq/Wk/Wv are
column-sharded, Wo row-sharded; each core emits a transposed partial output
outT = (context_g @ Wo_g)^T (bf16) which the host transposes and sums per batch.

v2 design notes (vs the original baseline):
- host passes x pre-transposed (xT, bf16): the QKV matmul consumes xT chunks as
  lhsT directly -> no on-device x transposes / casts / PSUM evacuations.
- L2 norm: sum-of-squares via ACT Square with accum_out, then
  rsqrt = exp(-0.5*ln(x)) on ACT. Softmax denominators: 1/x = exp(-ln(x)).
  All ACT functions (exp/ln/square/copy) live in one table set
  (natural_log_exp_and_others) -> single ACT_TABLE_LOAD for the whole kernel.
- RoPE as 4 tensor_tensor ops using host-packed [cos|sin] and [sin|cos] tables.
- sliding-window masks are 3 constant 128x128 tiles (diag triangle, strict
  complement, and the t=2 strict|global variant); the kt==t-1 k-tile is fully
  unmasked and global rows for t>=3 are fully unmasked (no mask DMA stream).
- denominator broadcast via a K=1 ones matmul on the PE (no gpsimd).
- phase A/B/C interleaved per-tile so PE/ACT/DVE overlap across phases.
"""

import sys

for _p in (
    "/opt/trn_rl_repo",
    "/root/.axon_site",
    "/root/.axon_site/_ro/pypackages",
    "/root/.axon_site/_ro/trn_rl_repo",
):
    if _p not in sys.path:
        sys.path.insert(0, _p)

from contextlib import ExitStack

import numpy as np

import concourse.bass as bass  # noqa: F401  (registers engine classes)
import concourse.tile as tile
from concourse import bacc, mybir
from concourse.bass_utils import run_bass_kernel_spmd
from concourse.masks import make_identity

B, S, DM = 2, 2048, 1024
NH, NKV, DH = 16, 4, 64
HPC = 4  # q heads per core (one full GQA group)
WINDOW, NGLOB = 256, 4
SCALE = 1.0 / np.sqrt(DH)
CAP = 15.0
EPS = 1e-8
P = 128
NT = S // P  # 16 sequence tiles
G = HPC + 1  # 4 q heads + 1 k head share L2norm/RoPE processing
F32 = mybir.dt.float32
BF16 = mybir.dt.bfloat16
MULT = mybir.AluOpType.mult
AF = mybir.ActivationFunctionType


def _build_kernel(ctx, tc, d):
    nc = tc.nc

    consts = ctx.enter_context(tc.tile_pool(name="consts", bufs=1))
    ident = consts.tile([P, P], F32)
    make_identity(nc, ident[:])
    ident_bf = consts.tile([P, P], BF16)
    nc.vector.tensor_copy(ident_bf[:], ident[:])

    wqkv_sb = consts.tile([P, 8, 384], BF16)
    nc.sync.dma_start(wqkv_sb[:], d["wqkv"].rearrange("(c p) n -> p c n", p=P))
    wo_sb = consts.tile([P, 2, DM], BF16)
    nc.sync.dma_start(wo_sb[:], d["wo"].rearrange("(c p) n -> p c n", p=P))
    cs1_sb = consts.tile([P, NT, 64], BF16)
    nc.sync.dma_start(cs1_sb[:], d["cs1"].rearrange("(t p) n -> p t n", p=P))
    cs2_sb = consts.tile([P, NT, 64], BF16)
    nc.sync.dma_start(cs2_sb[:], d["cs2"].rearrange("(t p) n -> p t n", p=P))
    masks_sb = consts.tile([P, 3, P], BF16)
    nc.sync.dma_start(masks_sb[:], d["masks"].rearrange("p (j q) -> p j q", j=3))
    ones1 = consts.tile([P, 1], F32)
    nc.vector.memset(ones1[:], 1.0)

    # persistent per-s-chunk tensors
    qt_pool = ctx.enter_context(tc.tile_pool(name="qt", bufs=NT))
    kt_pool = ctx.enter_context(tc.tile_pool(name="kt", bufs=NT))
    v_pool = ctx.enter_context(tc.tile_pool(name="v", bufs=NT))
    ctx_pool = ctx.enter_context(tc.tile_pool(name="ctx", bufs=8))

    xt_pool = ctx.enter_context(tc.tile_pool(name="xt", bufs=3))
    work = ctx.enter_context(tc.tile_pool(name="work", bufs=4))
    attn = ctx.enter_context(tc.tile_pool(name="attn", bufs=6))
    outp = ctx.enter_context(tc.tile_pool(name="outp", bufs=4))

    ps_t = ctx.enter_context(tc.tile_pool(name="ps_t", bufs=2, space="PSUM"))
    ps_mm = ctx.enter_context(tc.tile_pool(name="ps_mm", bufs=2, space="PSUM"))
    ps_sc = ctx.enter_context(tc.tile_pool(name="ps_sc", bufs=2, space="PSUM"))
    ps_cx = ctx.enter_context(tc.tile_pool(name="ps_cx", bufs=2, space="PSUM"))

    qtiles, ktiles, vtiles = [], [], []
    ctxt = [[None] * 4, [None] * 4]
    for c in range(2):
        for sc in range(4):
            ctile = ctx_pool.tile([P, 512], BF16, name=f"ctx_{c}_{sc}", tag="ctx")
            ctxt[c][sc] = ctile

    def phase_a(i):
        xt = xt_pool.tile([P, 8, P], BF16, name=f"xt_{i}", tag="xt")
        nc.sync.dma_start(
            xt[:],
            d["xT"][:, P * i : P * (i + 1)].rearrange("(c p) s -> p c s", p=P),
        )

        pq = ps_mm.tile([P, 384], F32, name=f"pqkv_{i}", tag="mm")
        for mj in range(8):
            nc.tensor.matmul(
                pq[:],
                lhsT=xt[:, mj, :],
                rhs=wqkv_sb[:, mj, :],
                start=(mj == 0),
                stop=(mj == 7),
            )

        # L2 norms over d for the 4 q heads + 1 k head (first 320 cols):
        # per-group sum-of-squares via ACT Square + accum_out, then
        # rsqrt via exp(-0.5*ln(x)) (same ACT table set as Exp).
        ssq = work.tile([P, G * DH], F32, tag="ssq")
        red = work.tile([P, G], F32, tag="red")
        for g in range(G):
            nc.scalar.activation(
                ssq[:, DH * g : DH * (g + 1)],
                pq[:, DH * g : DH * (g + 1)],
                AF.Square,
                accum_out=red[:, g : g + 1],
            )
        lgs = work.tile([P, G], F32, tag="lgs")
        nc.scalar.activation(lgs[:], red[:], AF.Ln)
        rcn = work.tile([P, G], F32, tag="rcn")
        nc.scalar.activation(rcn[:], lgs[:], AF.Exp, scale=-0.5)
        qkn = work.tile([P, G * DH], BF16, tag="qkn")
        nc.vector.tensor_tensor(
            qkn[:].rearrange("p (g n) -> p g n", g=G),
            pq[:, 0 : G * DH].rearrange("p (g n) -> p g n", g=G),
            rcn[:].unsqueeze(-1).broadcast_to([P, G, DH]),
            op=MULT,
        )

        # v (+ ones column for softmax sums)
        vt_i = v_pool.tile([P, 65], BF16, name=f"v_{i}", tag="v")
        nc.scalar.copy(vt_i[:, 64:65], ones1[:])
        nc.scalar.copy(vt_i[:, 0:64], pq[:, 320:384])
        vtiles.append(vt_i)

        # RoPE: rp[.., 0:32] = a*cos - b*sin ; rp[.., 32:64] = a*sin + b*cos
        # t_ac = qkn * [cos|sin], t_as = qkn * [sin|cos]
        qv = qkn[:].rearrange("p (g n) -> p g n", g=G)
        t_ac = work.tile([P, G * DH], BF16, tag="tac")
        t_as = work.tile([P, G * DH], BF16, tag="tas")
        nc.vector.tensor_tensor(
            t_ac[:].rearrange("p (g n) -> p g n", g=G),
            qv,
            cs1_sb[:, i, :].unsqueeze(1).broadcast_to([P, G, DH]),
            op=MULT,
        )
        nc.vector.tensor_tensor(
            t_as[:].rearrange("p (g n) -> p g n", g=G),
            qv,
            cs2_sb[:, i, :].unsqueeze(1).broadcast_to([P, G, DH]),
            op=MULT,
        )
        rp = work.tile([P, G * DH], BF16, tag="rp")
        rv = rp[:].rearrange("p (g n) -> p g n", g=G)
        acv = t_ac[:].rearrange("p (g n) -> p g n", g=G)
        asv = t_as[:].rearrange("p (g n) -> p g n", g=G)
        nc.vector.tensor_sub(rv[:, :, 0:32], acv[:, :, 0:32], acv[:, :, 32:64])
        nc.vector.tensor_add(rv[:, :, 32:64], asv[:, :, 0:32], asv[:, :, 32:64])

        # transpose q (2x 128-col blocks = 4 heads) and k (64 cols)
        qt_i = qt_pool.tile([64, HPC * P], BF16, name=f"qt_{i}", tag="qt")
        for hp in range(2):
            ptq = ps_t.tile([P, P], BF16, name=f"ptq_{i}_{hp}", tag="t")
            nc.tensor.transpose(ptq[:], rp[:, P * hp : P * (hp + 1)], ident_bf[:])
            nc.scalar.copy(qt_i[:, (2 * hp) * P : (2 * hp) * P + P], ptq[0:64, :])
            nc.vector.tensor_copy(
                qt_i[:, (2 * hp + 1) * P : (2 * hp + 1) * P + P], ptq[64:128, :]
            )
        ptk = ps_t.tile([P, P], BF16, name=f"ptk_{i}", tag="t")
        nc.tensor.transpose(ptk[0:64, :], rp[:, 256:320], ident_bf[:])
        kt_i = kt_pool.tile([64, P], BF16, name=f"kt_{i}", tag="kt")
        nc.scalar.copy(kt_i[:], ptk[0:64, :])
        qtiles.append(qt_i)
        ktiles.append(kt_i)

    def phase_b(t):
        kts = list(range(max(0, t - 2), t + 1))
        qrhs = qtiles[t][:].rearrange("p (h q) -> p h q", h=HPC)
        pcx = ps_cx.tile([65, 512], F32, name=f"pcx_{t}", tag="cx")
        n_ctx = len(kts) + (1 if t >= 3 else 0)
        ci = 0
        for kt in kts:
            ps = ps_sc.tile([P, 512], F32, name=f"psc_{t}_{kt}", tag="sc")
            nc.tensor.matmul(
                ps[:], lhsT=ktiles[kt][:], rhs=qrhs, start=True, stop=True
            )
            ex = attn.tile([P, 512], BF16, tag="ex")
            nc.scalar.activation(ex[:], ps[:], AF.Exp, scale=SCALE)
            if kt == t:
                mk = 0  # diagonal: p <= q
            elif kt == t - 1:
                mk = None  # fully inside the window: no mask
            elif t == 2 and kt == 0:
                mk = 2  # strict complement + global rows
            else:
                mk = 1  # strict complement: p > q
            if mk is not None:
                em = attn.tile([P, 512], BF16, tag="em")
                nc.vector.tensor_tensor(
                    em[:].rearrange("p (h q) -> p h q", h=HPC),
                    ex[:].rearrange("p (h q) -> p h q", h=HPC),
                    masks_sb[:, mk, :].unsqueeze(1).broadcast_to([P, HPC, P]),
                    op=MULT,
                )
                rhs_t = em
            else:
                rhs_t = ex
            nc.tensor.matmul(
                pcx[:],
                lhsT=vtiles[kt][:],
                rhs=rhs_t[:],
                start=(ci == 0),
                stop=(ci == n_ctx - 1),
            )
            ci += 1

        if t >= 3:
            # global rows (k < 4): fully unmasked for t >= 3
            psg = ps_sc.tile([4, 512], F32, name=f"psg_{t}", tag="sc")
            nc.tensor.matmul(
                psg[:], lhsT=ktiles[0][:, 0:4], rhs=qrhs, start=True, stop=True
            )
            exg = attn.tile([4, 512], BF16, tag="exg")
            nc.scalar.activation(exg[:], psg[:], AF.Exp, scale=SCALE)
            nc.tensor.matmul(
                pcx[:],
                lhsT=vtiles[0][0:4, :],
                rhs=exg[:],
                start=False,
                stop=True,
            )

        # softmax denominators (row 64 of pcx): 1/x = exp(-ln x) on ACT,
        # then broadcast to 64 partitions on the (otherwise idle) gpsimd so
        # the normalize TT reads only one PSUM operand (pcx).
        dn = attn.tile([1, 512], F32, tag="dn")
        nc.scalar.copy(dn[:], pcx[64:65, :])
        lg = attn.tile([1, 512], F32, tag="lg")
        nc.scalar.activation(lg[:], dn[:], AF.Ln)
        rcb = attn.tile([1, 512], F32, tag="rcb")
        nc.scalar.activation(rcb[:], lg[:], AF.Exp, scale=-1.0)
        rb = attn.tile([64, 512], F32, tag="rb")
        nc.gpsimd.partition_broadcast(rb[:], rcb[:])

        sc_, qoff = t // 4, (t % 4) * P
        for h in range(HPC):
            c, p0 = h // 2, 64 * (h % 2)
            nc.vector.tensor_tensor(
                ctxt[c][sc_][p0 : p0 + 64, qoff : qoff + P],
                pcx[0:64, h * P : (h + 1) * P],
                rb[:, h * P : (h + 1) * P],
                op=MULT,
            )

    def phase_c(sc):
        for mo in range(8):
            po = ps_mm.tile([P, 512], F32, name=f"po_{sc}_{mo}", tag="mm")
            for c in range(2):
                nc.tensor.matmul(
                    po[:],
                    lhsT=wo_sb[:, c, P * mo : P * (mo + 1)],
                    rhs=ctxt[c][sc][:],
                    start=(c == 0),
                    stop=(c == 1),
                )
            ob = outp.tile([P, 512], BF16, tag="ob")
            if mo % 2 == 0:
                nc.scalar.copy(ob[:], po[:])
            else:
                nc.vector.tensor_copy(ob[:], po[:])
            nc.sync.dma_start(
                d["outT"][P * mo : P * (mo + 1), 512 * sc : 512 * (sc + 1)], ob[:]
            )

    for t in range(NT):
        phase_a(t)
        phase_b(t)
        if t % 4 == 3:
            phase_c(t // 4)


def build_program():
    nc = bacc.Bacc("TRN2", target_bir_lowering=False, debug=False, num_devices=8)
    d = {}
    d["xT"] = nc.dram_tensor("xT", [DM, S], BF16, kind="ExternalInput").ap()
    d["wqkv"] = nc.dram_tensor("wqkv", [DM, 384], BF16, kind="ExternalInput").ap()
    d["wo"] = nc.dram_tensor("wo", [256, DM], BF16, kind="ExternalInput").ap()
    d["cs1"] = nc.dram_tensor("cs1", [S, 64], BF16, kind="ExternalInput").ap()
    d["cs2"] = nc.dram_tensor("cs2", [S, 64], BF16, kind="ExternalInput").ap()
    d["masks"] = nc.dram_tensor("masks", [P, 3 * P], BF16, kind="ExternalInput").ap()
    d["outT"] = nc.dram_tensor("outT", [DM, S], BF16, kind="ExternalOutput").ap()
    with tile.TileContext(nc) as tc, ExitStack() as ctx:
        _build_kernel(ctx, tc, d)
    nc.compile()
    return nc


def make_masks(mask_np):
    """Build the 3 constant [k, q] mask tiles (diag tri, strict, t=2 variant)
    from the caller mask combined with the sliding-window|global pattern."""
    mask_np = np.asarray(mask_np).astype(bool)
    q = np.arange(S)[:, None]
    k = np.arange(S)[None, :]
    wmask = ((k <= q) & (k > q - WINDOW)) | (k < NGLOB)
    combT = (mask_np[0, 0] & wmask).T.astype(np.float32)  # [k, q]
    tri = combT[5 * P : 6 * P, 5 * P : 6 * P]  # t=5, kt=5 (diag)
    strict = combT[3 * P : 4 * P, 5 * P : 6 * P]  # t=5, kt=3 (strict)
    t2 = combT[0:P, 2 * P : 3 * P]  # t=2, kt=0 (strict | global)
    return np.stack([tri, strict, t2], axis=1)  # [P, 3, P]


def make_in_maps(x, cos, sin, mask, Wq, Wk, Wv, Wo):
    import ml_dtypes

    bf = ml_dtypes.bfloat16
    x = np.asarray(x, np.float32)
    cos = np.asarray(cos, np.float32)
    sin = np.asarray(sin, np.float32)
    Wq, Wk, Wv, Wo = (np.asarray(a, np.float32).astype(bf) for a in (Wq, Wk, Wv, Wo))
    cs1 = np.concatenate([cos, sin], axis=1).astype(bf)  # [S, 64]
    cs2 = np.concatenate([sin, cos], axis=1).astype(bf)
    masks = make_masks(mask).reshape(P, 3 * P).astype(bf)
    xT = [np.ascontiguousarray(x[b].T.astype(bf)) for b in range(B)]
    in_maps = []
    for c in range(8):
        b, g = divmod(c, 4)
        wqkv = np.concatenate(
            [
                Wq[:, 256 * g : 256 * (g + 1)],
                Wk[:, 64 * g : 64 * (g + 1)],
                Wv[:, 64 * g : 64 * (g + 1)],
            ],
            axis=1,
        )
        in_maps.append(
            {
                "xT": xT[b],
                "wqkv": np.ascontiguousarray(wqkv),
                "wo": np.ascontiguousarray(Wo[256 * g : 256 * (g + 1), :]),
                "cs1": cs1,
                "cs2": cs2,
                "masks": masks,
            }
        )
    return in_maps


_PROGRAM = None


def _get_program():
    global _PROGRAM
    if _PROGRAM is None:
        _PROGRAM = build_program()
    return _PROGRAM


def kernel(x, cos, sin, mask, Wq, Wk, Wv, Wo, _trace=False, _trace_kwargs=None):
    nc = _get_program()
    in_maps = make_in_maps(x, cos, sin, mask, Wq, Wk, Wv, Wo)
    res = run_bass_kernel_spmd(
        nc, in_maps, list(range(8)), trace=_trace, **(_trace_kwargs or {})
    )
    out = np.zeros((B, S, DM), np.float32)
    for c in range(8):
        out[c // 4] += np.asarray(res.results[c]["outT"], dtype=np.float32).T
    if _trace:
        kernel._last_results = res
    return out


# revision 5
# speedup vs baseline: 2.0347x; 1.8747x over previous
"""Trainium2 Bass kernel for GroupedQueryAttention (sparse sliding-window + global).

Sharding: 8 cores = 2 (batch) x 4 (GQA groups). Core c handles batch c//4 and
kv-head g=c%4 together with its 4 query heads (heads 4g..4g+3). Wq/Wk/Wv are
column-sharded, Wo row-sharded; each core emits a transposed partial output
outT = (context_g @ Wo_g)^T (bf16) which the host transposes and sums per batch.

v3 design notes:
- host passes x pre-transposed (xT, bf16): the QKV matmul consumes xT chunks as
  lhsT directly -> no on-device x transposes / casts / PSUM evacuations.
- phase A1 (all tiles): QKV matmul, sum-of-squares, evacuate qkv to SBUF bf16.
  Then ONE batched Ln + Exp(-0.5) pair computes every L2-norm rsqrt at once:
  the ACT table-set chooser is greedy (exp->exp_and_others, ln->natural_log),
  so interleaving Ln with Exp per-tile would thrash ACT_TABLE_LOADs (~2.7us
  each). Batching leaves 3 table loads total for the whole kernel.
- phase A2/B/C interleaved per-tile: normalize+RoPE+transposes, then banded
  attention, with the output projection for each 512-chunk as it completes.
- softmax denominators via DVE reciprocal_approx_fast (no ACT table), then
  gpsimd partition_broadcast; normalize reads only one PSUM operand.
- sliding-window masks are 3 constant 128x128 tiles (diag triangle, strict
  complement, and the t=2 strict|global variant); the kt==t-1 k-tile is fully
  unmasked and global rows for t>=3 are fully unmasked (no mask DMA stream).
- RoPE as 4 tensor_tensor ops using host-packed [cos|sin] and [sin|cos] tables.
"""

import sys

for _p in (
    "/opt/trn_rl_repo",
    "/root/.axon_site",
    "/root/.axon_site/_ro/pypackages",
    "/root/.axon_site/_ro/trn_rl_repo",
):
    if _p not in sys.path:
        sys.path.insert(0, _p)

from contextlib import ExitStack

import numpy as np

import concourse.bass as bass  # noqa: F401  (registers engine classes)
import concourse.tile as tile
from concourse import bacc, mybir
from concourse.bass_utils import run_bass_kernel_spmd
from concourse.masks import make_identity

B, S, DM = 2, 2048, 1024
NH, NKV, DH = 16, 4, 64
HPC = 4  # q heads per core (one full GQA group)
WINDOW, NGLOB = 256, 4
SCALE = 1.0 / np.sqrt(DH)
CAP = 15.0
EPS = 1e-8
P = 128
NT = S // P  # 16 sequence tiles
G = HPC + 1  # 4 q heads + 1 k head share L2norm/RoPE processing
F32 = mybir.dt.float32
BF16 = mybir.dt.bfloat16
MULT = mybir.AluOpType.mult
AF = mybir.ActivationFunctionType


def _build_kernel(ctx, tc, d):
    nc = tc.nc

    consts = ctx.enter_context(tc.tile_pool(name="consts", bufs=1))
    ident = consts.tile([P, P], F32)
    make_identity(nc, ident[:])
    ident_bf = consts.tile([P, P], BF16)
    nc.vector.tensor_copy(ident_bf[:], ident[:])

    wqkv_sb = consts.tile([P, 8, 384], BF16)
    nc.sync.dma_start(wqkv_sb[:], d["wqkv"].rearrange("(c p) n -> p c n", p=P))
    wo_sb = consts.tile([P, 2, DM], BF16)
    nc.sync.dma_start(wo_sb[:], d["wo"].rearrange("(c p) n -> p c n", p=P))
    cs1_sb = consts.tile([P, NT, 64], BF16)
    nc.sync.dma_start(cs1_sb[:], d["cs1"].rearrange("(t p) n -> p t n", p=P))
    cs2_sb = consts.tile([P, NT, 64], BF16)
    nc.sync.dma_start(cs2_sb[:], d["cs2"].rearrange("(t p) n -> p t n", p=P))
    masks_sb = consts.tile([P, 3, P], BF16)
    nc.sync.dma_start(masks_sb[:], d["masks"].rearrange("p (j q) -> p j q", j=3))

    # persistent tensors
    qkv_pool = ctx.enter_context(tc.tile_pool(name="qkv", bufs=NT))
    qt_pool = ctx.enter_context(tc.tile_pool(name="qt", bufs=NT))
    kt_pool = ctx.enter_context(tc.tile_pool(name="kt", bufs=NT))
    ctx_pool = ctx.enter_context(tc.tile_pool(name="ctx", bufs=8))
    norm_pool = ctx.enter_context(tc.tile_pool(name="norm", bufs=1))

    xt_pool = ctx.enter_context(tc.tile_pool(name="xt", bufs=3))
    work = ctx.enter_context(tc.tile_pool(name="work", bufs=4))
    attn = ctx.enter_context(tc.tile_pool(name="attn", bufs=6))
    outp = ctx.enter_context(tc.tile_pool(name="outp", bufs=4))

    ps_t = ctx.enter_context(tc.tile_pool(name="ps_t", bufs=2, space="PSUM"))
    ps_mm = ctx.enter_context(tc.tile_pool(name="ps_mm", bufs=2, space="PSUM"))
    ps_sc = ctx.enter_context(tc.tile_pool(name="ps_sc", bufs=2, space="PSUM"))
    ps_cx = ctx.enter_context(tc.tile_pool(name="ps_cx", bufs=2, space="PSUM"))

    red_all = norm_pool.tile([P, NT, G], F32)
    lgs_all = norm_pool.tile([P, NT, G], F32)
    rcn_all = norm_pool.tile([P, NT, G], BF16)

    qkvtiles, qtiles, ktiles = [], [], []
    ctxt = [[None] * 4, [None] * 4]
    for c in range(2):
        for sc in range(4):
            ctile = ctx_pool.tile([P, 512], BF16, name=f"ctx_{c}_{sc}", tag="ctx")
            ctxt[c][sc] = ctile

    def phase_a1(i):
        xt = xt_pool.tile([P, 8, P], BF16, name=f"xt_{i}", tag="xt")
        nc.sync.dma_start(
            xt[:],
            d["xT"][:, P * i : P * (i + 1)].rearrange("(c p) s -> p c s", p=P),
        )
        pq = ps_mm.tile([P, 384], F32, name=f"pqkv_{i}", tag="mm")
        for mj in range(8):
            nc.tensor.matmul(
                pq[:],
                lhsT=xt[:, mj, :],
                rhs=wqkv_sb[:, mj, :],
                start=(mj == 0),
                stop=(mj == 7),
            )
        # sum of squares per head group (for the batched rsqrt later)
        ssq = work.tile([P, G * DH], F32, tag="ssq")
        nc.scalar.activation(ssq[:], pq[:, 0 : G * DH], AF.Square)
        nc.vector.tensor_reduce(
            red_all[:, i, :],
            ssq[:].rearrange("p (g n) -> p g n", g=G),
            axis=mybir.AxisListType.X,
            op=mybir.AluOpType.add,
        )
        # evacuate qkv to SBUF (bf16); col 384 is the ones column for the
        # softmax-denominator trick (v is cols 320:385 as the ctx lhsT).
        qkv_i = qkv_pool.tile([P, 385], BF16, name=f"qkv_{i}", tag="qkv")
        nc.scalar.copy(qkv_i[:, 0:384], pq[:])
        nc.vector.memset(qkv_i[:, 384:385], 1.0)
        qkvtiles.append(qkv_i)

    def norm_barrier():
        # one Ln + one Exp(-0.5 * .) over all tiles' sums of squares:
        # rsqrt(x) = exp(-0.5*ln(x)). Costs 2 ACT table switches, total.
        nc.scalar.activation(lgs_all[:], red_all[:], AF.Ln)
        nc.scalar.activation(rcn_all[:], lgs_all[:], AF.Exp, scale=-0.5)

    def phase_a2(i):
        qkv_i = qkvtiles[i]
        qkn = work.tile([P, G * DH], BF16, tag="qkn")
        nc.vector.tensor_tensor(
            qkn[:].rearrange("p (g n) -> p g n", g=G),
            qkv_i[:, 0 : G * DH].rearrange("p (g n) -> p g n", g=G),
            rcn_all[:, i, :].unsqueeze(-1).broadcast_to([P, G, DH]),
            op=MULT,
        )

        # RoPE: rp[.., 0:32] = a*cos - b*sin ; rp[.., 32:64] = a*sin + b*cos
        qv = qkn[:].rearrange("p (g n) -> p g n", g=G)
        t_ac = work.tile([P, G * DH], BF16, tag="tac")
        t_as = work.tile([P, G * DH], BF16, tag="tas")
        nc.vector.tensor_tensor(
            t_ac[:].rearrange("p (g n) -> p g n", g=G),
            qv,
            cs1_sb[:, i, :].unsqueeze(1).broadcast_to([P, G, DH]),
            op=MULT,
        )
        nc.vector.tensor_tensor(
            t_as[:].rearrange("p (g n) -> p g n", g=G),
            qv,
            cs2_sb[:, i, :].unsqueeze(1).broadcast_to([P, G, DH]),
            op=MULT,
        )
        rp = work.tile([P, G * DH], BF16, tag="rp")
        rv = rp[:].rearrange("p (g n) -> p g n", g=G)
        acv = t_ac[:].rearrange("p (g n) -> p g n", g=G)
        asv = t_as[:].rearrange("p (g n) -> p g n", g=G)
        nc.vector.tensor_sub(rv[:, :, 0:32], acv[:, :, 0:32], acv[:, :, 32:64])
        nc.vector.tensor_add(rv[:, :, 32:64], asv[:, :, 0:32], asv[:, :, 32:64])

        # transpose q (2x 128-col blocks = 4 heads) and k (64 cols)
        qt_i = qt_pool.tile([64, HPC * P], BF16, name=f"qt_{i}", tag="qt")
        for hp in range(2):
            ptq = ps_t.tile([P, P], BF16, name=f"ptq_{i}_{hp}", tag="t")
            nc.tensor.transpose(ptq[:], rp[:, P * hp : P * (hp + 1)], ident_bf[:])
            nc.scalar.copy(qt_i[:, (2 * hp) * P : (2 * hp) * P + P], ptq[0:64, :])
            nc.vector.tensor_copy(
                qt_i[:, (2 * hp + 1) * P : (2 * hp + 1) * P + P], ptq[64:128, :]
            )
        ptk = ps_t.tile([P, P], BF16, name=f"ptk_{i}", tag="t")
        nc.tensor.transpose(ptk[0:64, :], rp[:, 256:320], ident_bf[:])
        kt_i = kt_pool.tile([64, P], BF16, name=f"kt_{i}", tag="kt")
        nc.scalar.copy(kt_i[:], ptk[0:64, :])
        qtiles.append(qt_i)
        ktiles.append(kt_i)

    def phase_b(t):
        kts = list(range(max(0, t - 2), t + 1))
        qrhs = qtiles[t][:].rearrange("p (h q) -> p h q", h=HPC)
        pcx = ps_cx.tile([65, 512], F32, name=f"pcx_{t}", tag="cx")
        n_ctx = len(kts) + (1 if t >= 3 else 0)
        ci = 0
        for kt in kts:
            ps = ps_sc.tile([P, 512], F32, name=f"psc_{t}_{kt}", tag="sc")
            nc.tensor.matmul(
                ps[:], lhsT=ktiles[kt][:], rhs=qrhs, start=True, stop=True
            )
            ex = attn.tile([P, 512], BF16, tag="ex")
            nc.scalar.activation(ex[:], ps[:], AF.Exp, scale=SCALE)
            if kt == t:
                mk = 0  # diagonal: p <= q
            elif kt == t - 1:
                mk = None  # fully inside the window: no mask
            elif t == 2 and kt == 0:
                mk = 2  # strict complement + global rows
            else:
                mk = 1  # strict complement: p > q
            if mk is not None:
                em = attn.tile([P, 512], BF16, tag="em")
                nc.vector.tensor_tensor(
                    em[:].rearrange("p (h q) -> p h q", h=HPC),
                    ex[:].rearrange("p (h q) -> p h q", h=HPC),
                    masks_sb[:, mk, :].unsqueeze(1).broadcast_to([P, HPC, P]),
                    op=MULT,
                )
                rhs_t = em
            else:
                rhs_t = ex
            nc.tensor.matmul(
                pcx[:],
                lhsT=qkvtiles[kt][:, 320:385],
                rhs=rhs_t[:],
                start=(ci == 0),
                stop=(ci == n_ctx - 1),
            )
            ci += 1

        if t >= 3:
            # global rows (k < 4): fully unmasked for t >= 3
            psg = ps_sc.tile([4, 512], F32, name=f"psg_{t}", tag="sc")
            nc.tensor.matmul(
                psg[:], lhsT=ktiles[0][:, 0:4], rhs=qrhs, start=True, stop=True
            )
            exg = attn.tile([4, 512], BF16, tag="exg")
            nc.scalar.activation(exg[:], psg[:], AF.Exp, scale=SCALE)
            nc.tensor.matmul(
                pcx[:],
                lhsT=qkvtiles[0][0:4, 320:385],
                rhs=exg[:],
                start=False,
                stop=True,
            )

        # softmax denominators (row 64 of pcx): reciprocal on DVE (no ACT
        # table), broadcast to 64 partitions on gpsimd so the normalize TT
        # reads only one PSUM operand (pcx).
        dn = attn.tile([1, 512], F32, tag="dn")
        nc.scalar.copy(dn[:], pcx[64:65, :])
        rcb = attn.tile([1, 512], F32, tag="rcb")
        nc.vector.reciprocal_approx_fast(rcb[:], dn[:])
        rb = attn.tile([64, 512], F32, tag="rb")
        nc.gpsimd.partition_broadcast(rb[:], rcb[:])

        sc_, qoff = t // 4, (t % 4) * P
        for h in range(HPC):
            c, p0 = h // 2, 64 * (h % 2)
            nc.vector.tensor_tensor(
                ctxt[c][sc_][p0 : p0 + 64, qoff : qoff + P],
                pcx[0:64, h * P : (h + 1) * P],
                rb[:, h * P : (h + 1) * P],
                op=MULT,
            )

    def phase_c(sc):
        for mo in range(8):
            po = ps_mm.tile([P, 512], F32, name=f"po_{sc}_{mo}", tag="mm")
            for c in range(2):
                nc.tensor.matmul(
                    po[:],
                    lhsT=wo_sb[:, c, P * mo : P * (mo + 1)],
                    rhs=ctxt[c][sc][:],
                    start=(c == 0),
                    stop=(c == 1),
                )
            ob = outp.tile([P, 512], BF16, tag="ob")
            if mo % 2 == 0:
                nc.scalar.copy(ob[:], po[:])
            else:
                nc.vector.tensor_copy(ob[:], po[:])
            nc.sync.dma_start(
                d["outT"][P * mo : P * (mo + 1), 512 * sc : 512 * (sc + 1)], ob[:]
            )

    for t in range(NT):
        phase_a1(t)
    norm_barrier()
    for t in range(NT):
        phase_a2(t)
        phase_b(t)
        if t % 4 == 3:
            phase_c(t // 4)


def build_program():
    nc = bacc.Bacc("TRN2", target_bir_lowering=False, debug=False, num_devices=8)
    d = {}
    d["xT"] = nc.dram_tensor("xT", [DM, S], BF16, kind="ExternalInput").ap()
    d["wqkv"] = nc.dram_tensor("wqkv", [DM, 384], BF16, kind="ExternalInput").ap()
    d["wo"] = nc.dram_tensor("wo", [256, DM], BF16, kind="ExternalInput").ap()
    d["cs1"] = nc.dram_tensor("cs1", [S, 64], BF16, kind="ExternalInput").ap()
    d["cs2"] = nc.dram_tensor("cs2", [S, 64], BF16, kind="ExternalInput").ap()
    d["masks"] = nc.dram_tensor("masks", [P, 3 * P], BF16, kind="ExternalInput").ap()
    d["outT"] = nc.dram_tensor("outT", [DM, S], BF16, kind="ExternalOutput").ap()
    with tile.TileContext(nc) as tc, ExitStack() as ctx:
        _build_kernel(ctx, tc, d)
    nc.compile()
    return nc


def make_masks(mask_np):
    """Build the 3 constant [k, q] mask tiles (diag tri, strict, t=2 variant)
    from the caller mask combined with the sliding-window|global pattern."""
    mask_np = np.asarray(mask_np).astype(bool)
    q = np.arange(S)[:, None]
    k = np.arange(S)[None, :]
    wmask = ((k <= q) & (k > q - WINDOW)) | (k < NGLOB)
    combT = (mask_np[0, 0] & wmask).T.astype(np.float32)  # [k, q]
    tri = combT[5 * P : 6 * P, 5 * P : 6 * P]  # t=5, kt=5 (diag)
    strict = combT[3 * P : 4 * P, 5 * P : 6 * P]  # t=5, kt=3 (strict)
    t2 = combT[0:P, 2 * P : 3 * P]  # t=2, kt=0 (strict | global)
    return np.stack([tri, strict, t2], axis=1)  # [P, 3, P]


def make_in_maps(x, cos, sin, mask, Wq, Wk, Wv, Wo):
    import ml_dtypes

    bf = ml_dtypes.bfloat16
    x = np.asarray(x, np.float32)
    cos = np.asarray(cos, np.float32)
    sin = np.asarray(sin, np.float32)
    Wq, Wk, Wv, Wo = (np.asarray(a, np.float32).astype(bf) for a in (Wq, Wk, Wv, Wo))
    cs1 = np.concatenate([cos, sin], axis=1).astype(bf)  # [S, 64]
    cs2 = np.concatenate([sin, cos], axis=1).astype(bf)
    masks = make_masks(mask).reshape(P, 3 * P).astype(bf)
    xT = [np.ascontiguousarray(x[b].T.astype(bf)) for b in range(B)]
    in_maps = []
    for c in range(8):
        b, g = divmod(c, 4)
        wqkv = np.concatenate(
            [
                Wq[:, 256 * g : 256 * (g + 1)],
                Wk[:, 64 * g : 64 * (g + 1)],
                Wv[:, 64 * g : 64 * (g + 1)],
            ],
            axis=1,
        )
        in_maps.append(
            {
                "xT": xT[b],
                "wqkv": np.ascontiguousarray(wqkv),
                "wo": np.ascontiguousarray(Wo[256 * g : 256 * (g + 1), :]),
                "cs1": cs1,
                "cs2": cs2,
                "masks": masks,
            }
        )
    return in_maps


_PROGRAM = None


def _get_program():
    global _PROGRAM
    if _PROGRAM is None:
        _PROGRAM = build_program()
    return _PROGRAM


def kernel(x, cos, sin, mask, Wq, Wk, Wv, Wo, _trace=False, _trace_kwargs=None):
    nc = _get_program()
    in_maps = make_in_maps(x, cos, sin, mask, Wq, Wk, Wv, Wo)
    res = run_bass_kernel_spmd(
        nc, in_maps, list(range(8)), trace=_trace, **(_trace_kwargs or {})
    )
    out = np.zeros((B, S, DM), np.float32)
    for c in range(8):
        out[c // 4] += np.asarray(res.results[c]["outT"], dtype=np.float32).T
    if _trace:
        kernel._last_results = res
    return out
